# revision 24
# baseline (speedup 1.0000x reference)
"""Trainium2 Bass kernel for nn_BlockG (StyleGAN2-ish block G).

Reference math (per sample b):
    x_up  = up2(x)                 # bilinear x2, align_corners
    x1    = leaky(conv_demod(x_up, w, W1, A1, B1, n1))
    x2    = leaky(conv_demod(x1,  w, W2, A2, B2, n2))
    rgb   = up2(rgb) + leaky(conv_demod(x2, w, W3, A3, B3, n3))
    return (x2, rgb)

Key identities used on-device:
  conv(img, W*s[ci]*d[co]) = d[co] * conv(img*s[ci], W)  -> weights shared
  d = rsqrt(Wsq @ s^2 + eps),  Wsq[co,ci] = sum_kk W^2
  leaky(d*P) = d * max(P, 0.2*P)           (d > 0)
  B*noise enters PSUM as a K=1 matmul with lhsT = B/d = B*sqrt(v+eps)

Sharding: pure data-parallel, one sample per NeuronCore (batch 8 = 8 cores).
All matmuls run as float32r (FP22) at full PE rate.
"""

import numpy as np
from contextlib import ExitStack

import concourse.bass as bass
import concourse.bacc as bacc
import concourse.mybir as mybir
import concourse.tile as tile
from concourse.bass_utils import run_bass_kernel_spmd

F32 = mybir.dt.float32
F32R = mybir.dt.float32r

B_ = 8          # batch == n_cores
C = 512         # CIN == COUT == Z
NCH = 4         # channel chunks of 128
RGBC = 3
RIN = 32        # input spatial
ROUT = 64       # output spatial
PAD = 66        # padded spatial
NPIX = ROUT * ROUT   # 4096
NT = 8          # spatial tiles (8 rows x 64 cols = 512 px)
EPS = 1e-8
SLOPE = 0.2


# ---------------------------------------------------------------- upsample
def _up_coeffs():
    """Per-output-position lerp coefficients for 32->64 align_corners x2,
    replicating reference float32 arithmetic.

    even o=2m (m=1..31): out = x[m-1] + (x[m]-x[m-1]) * ce[m-1]
    odd  o=2m+1 (m=0..30): out = x[m]   + (x[m+1]-x[m]) * co[m]
    o=0 -> copy x[0]; o=63 -> copy x[31]
    """
    n, o = RIN, ROUT
    c = np.arange(o, dtype=np.float32) * np.float32((n - 1) / (o - 1))
    i0 = np.floor(c).astype(np.int64)
    t = (c - i0).astype(np.float32)
    ce = np.zeros(31, np.float32)
    co = np.zeros(31, np.float32)
    for m in range(1, 32):
        assert i0[2 * m] == m - 1, (m, i0[2 * m])
        ce[m - 1] = t[2 * m]
    for m in range(0, 31):
        assert i0[2 * m + 1] == m, (m, i0[2 * m + 1])
        co[m] = t[2 * m + 1]
    assert i0[0] == 0 and t[0] == 0.0
    # o=63: c may land exactly on 31 or just below; both reduce to ~x[31].
    assert i0[63] == 31 or (i0[63] == 30 and t[63] > 0.999999)
    return ce, co


def _emit_up2(nc, wpool, ce_sb, co_sb, src_ap, xh_ap, dst_ap, parts, eng=None):
    """Upsample src [parts,32,32] -> dst [parts,64,64] using xh [parts,64,32]
    scratch. dst/xh are arbitrary (possibly strided) SBUF APs."""
    v = eng if eng is not None else nc.vector
    # ---- H axis: src [p,32,32] -> xh [p,64,32]
    dh = wpool.tile([parts, 31, 32], F32, tag="w", name="up_dh")
    v.tensor_sub(dh[:], src_ap[:, 1:32, :], src_ap[:, 0:31, :])
    ceb = ce_sb[0:parts, :].unsqueeze(2).broadcast_to([parts, 31, 32])
    cob = co_sb[0:parts, :].unsqueeze(2).broadcast_to([parts, 31, 32])
    dm = wpool.tile([parts, 31, 32], F32, tag="w", name="up_dm")
    v.tensor_mul(dm[:], dh[:], ceb)
    v.tensor_add(xh_ap[:, 2:64:2, :], dm[:], src_ap[:, 0:31, :])
    dm2 = wpool.tile([parts, 31, 32], F32, tag="w", name="up_dm2")
    v.tensor_mul(dm2[:], dh[:], cob)
    v.tensor_add(xh_ap[:, 1:62:2, :], dm2[:], src_ap[:, 0:31, :])
    v.tensor_copy(xh_ap[:, 0:1, :], src_ap[:, 0:1, :])
    v.tensor_copy(xh_ap[:, 63:64, :], src_ap[:, 31:32, :])
    # ---- W axis: xh [p,64,32] -> dst [p,64,64], two row-halves
    for h2 in range(2):
        xv = xh_ap[:, 32 * h2:32 * h2 + 32, :]
        dv = dst_ap[:, 32 * h2:32 * h2 + 32, :]
        dw = wpool.tile([parts, 32, 31], F32, tag="w", name="up_dw")
        v.tensor_sub(dw[:], xv[:, :, 1:32], xv[:, :, 0:31])
        cwb = ce_sb[0:parts, :].unsqueeze(1).broadcast_to([parts, 32, 31])
        cob2 = co_sb[0:parts, :].unsqueeze(1).broadcast_to([parts, 32, 31])
        wm = wpool.tile([parts, 32, 31], F32, tag="w", name="up_wm")
        v.tensor_mul(wm[:], dw[:], cwb)
        v.tensor_add(dv[:, :, 2:64:2], wm[:], xv[:, :, 0:31])
        wm2 = wpool.tile([parts, 32, 31], F32, tag="w", name="up_wm2")
        v.tensor_mul(wm2[:], dw[:], cob2)
        v.tensor_add(dv[:, :, 1:62:2], wm2[:], xv[:, :, 0:31])
        v.tensor_copy(dv[:, :, 0:1], xv[:, :, 0:1])
        v.tensor_copy(dv[:, :, 63:64], xv[:, :, 31:32])


# ---------------------------------------------------------------- program
def _emit(ctx, tc, io):
    nc = tc.nc
    v, sc, te, sy = nc.vector, nc.scalar, nc.tensor, nc.sync

    # ---------------- pools
    pbuf = ctx.enter_context(tc.tile_pool(name="pbuf", bufs=1))
    P1 = [pbuf.tile([128, ROUT, PAD], F32R, tag=f"P1_{k}", name=f"P1_{k}")
          for k in range(NCH)]
    P2 = [pbuf.tile([128, ROUT, PAD], F32R, tag=f"P2_{k}", name=f"P2_{k}")
          for k in range(NCH)]
    wpool = ctx.enter_context(tc.tile_pool(name="wpool", bufs=14))
    wsqp = ctx.enter_context(tc.tile_pool(name="wsqp", bufs=2))
    sp = ctx.enter_context(tc.tile_pool(name="sp", bufs=1))     # unique tags
    sp2 = ctx.enter_context(tc.tile_pool(name="sp2", bufs=2))   # pipelined tags
    cps = ctx.enter_context(tc.tile_pool(name="cps", bufs=6, space="PSUM"))
    sps = ctx.enter_context(tc.tile_pool(name="sps", bufs=2, space="PSUM"))

    # ---------------- coefficients
    ce_sb = sp.tile([128, 31], F32, tag="ce", name="ce")
    co_sb = sp.tile([128, 31], F32, tag="co", name="co")
    sy.dma_start(out=ce_sb[:], in_=io["ce"])
    sy.dma_start(out=co_sb[:], in_=io["co"])
    eps_sb = sp.tile([128, 1], F32, tag="eps", name="eps")
    v.memset(eps_sb[:], EPS)

    # ---------------- styles for all 3 layers: s = A w + b, ssq = s^2
    wv_sb = []
    for k in range(NCH):
        t = sp.tile([128, 2], F32R, tag=f"wv{k}", name=f"wv{k}")
        sy.dma_start(out=t[:], in_=io["wv"][k * 128:(k + 1) * 128])
        wv_sb.append(t)

    s_sb = {}     # (L, m) -> [128,1]
    ssq_sb = {}   # (L, m) -> [128,1]
    for L, (at_name, ab_name) in enumerate(
            [("a1t", "a1b"), ("a2t", "a2b"), ("a3t", "a3b")], start=1):
        halves = []
        for h in range(2):
            ht = wpool.tile([128, 8, 128], F32R, tag="w", name=f"aT{L}_{h}")
            sy.dma_start(out=ht[:], in_=io[at_name][h])
            halves.append(ht)
        for m in range(NCH):
            ab = sp2.tile([128, 1], F32, tag="ab", name=f"ab{L}{m}")
            sy.dma_start(out=ab[:], in_=io[ab_name][m * 128:(m + 1) * 128])
            s_ps = cps.tile([128, 2], F32, tag="cps", name=f"s_ps{L}{m}")
            for k in range(NCH):
                b = k * 4 + m
                lhsT = halves[b // 8][:, b % 8, :].bitcast(F32R)
                te.matmul(s_ps[:], lhsT, wv_sb[k][:].bitcast(F32R),
                          start=(k == 0), stop=(k == NCH - 1))
            st = sp.tile([128, 1], F32, tag=f"s{L}_{m}", name=f"s{L}_{m}")
            sc.activation(st[:], s_ps[:, 0:1],
                          mybir.ActivationFunctionType.Identity,
                          bias=ab[:], scale=1.0)
            sq = sp.tile([128, 2], F32R, tag=f"ssq{L}_{m}", name=f"ssq{L}_{m}")
            sc.square(sq[:], st[:].broadcast_to([128, 2]))
            s_sb[(L, m)] = st
            ssq_sb[(L, m)] = sq

    # ---------------- conv jobs: layers 1&2 x 4 chunks, software-pipelined.
    # prep_dve(j): weight DMA + ACT squares + DVE reduces -> wsq[j]
    #   (emitted 2 jobs ahead, right after job j-2's first n-tile so the
    #    reduces land ahead of most drains in the DVE queue)
    # prep_pe(j): tiny v/vrow matmuls + sqrt/recip/bnd (emitted just before
    #    job j's conv so the PE hits them with inputs long ready)
    jobs = [(1, m) for m in range(NCH)] + [(2, m) for m in range(NCH)]
    WT = {1: "w1t", 2: "w2t"}
    NN = {1: "n1", 2: "n2"}
    BT = {1: "b1t", 2: "b2t"}
    SRC = {1: P1, 2: P2}
    DST = {1: P2, 2: P1}

    wm_j = {}
    wsq_j = {}

    def prep_dve(j):
        L, m = jobs[j]
        wm = []
        for k in range(NCH):
            t = wpool.tile([128, 9, 128], F32R, tag="w", name=f"w{L}m{m}k{k}")
            sy.dma_start(out=t[:], in_=io[WT[L]][m, k])
            wm.append(t)
        wsq = wsqp.tile([128, NCH, 128], F32R, tag="wsq", name=f"wsq{L}{m}")
        for k in range(NCH):
            sqt = wpool.tile([128, 9, 128], F32, tag="w", name=f"sq{L}{m}{k}")
            sc.square(sqt[:], wm[k][:])
            with nc.allow_low_precision("fp32r matmul operand"):
                v.tensor_reduce(wsq[:, k, :], sqt[:].transpose([0, 2, 1]),
                                axis=mybir.AxisListType.X,
                                op=mybir.AluOpType.add)
            # fold this layer's input-style scale into the conv weights
            # (per-partition ci scale; must happen after the squares)
            sc.mul(wm[k][:], wm[k][:], s_sb[(L, k)][:])
        wm_j[j] = wm
        wsq_j[j] = wsq

    def prep_pe(j):
        L, m = jobs[j]
        wsq = wsq_j[j]
        vcol = sps.tile([128, 2], F32, tag="sps", name=f"vc{L}{m}")
        for k in range(NCH):
            te.matmul(vcol[:], wsq[:, k, :].bitcast(F32R),
                      ssq_sb[(L, k)][:].bitcast(F32R),
                      start=(k == 0), stop=(k == NCH - 1))
        vrow = sps.tile([1, 128], F32, tag="sps", name=f"vr{L}{m}")
        for k in range(NCH):
            te.matmul(vrow[:], ssq_sb[(L, k)][:, 0:1].bitcast(F32R),
                      wsq[:, k, :].bitcast(F32R),
                      start=(k == 0), stop=(k == NCH - 1))
        sqv = sp2.tile([128, 1], F32, tag="sqv", name=f"sqv{L}{m}")
        sc.activation(sqv[:], vcol[:, 0:1], mybir.ActivationFunctionType.Sqrt,
                      bias=eps_sb[:], scale=1.0)
        d_m = sp2.tile([128, 1], F32, tag="d", name=f"d{L}{m}")
        v.reciprocal(d_m[:], sqv[:])
        sqvT = sp2.tile([1, 128], F32, tag="sqvT", name=f"sqvT{L}{m}")
        sc.activation(sqvT[:], vrow[:], mybir.ActivationFunctionType.Sqrt,
                      bias=eps_sb[0:1, :], scale=1.0)
        bsb = sp2.tile([1, 128], F32, tag="bsb", name=f"bsb{L}{m}")
        sy.dma_start(out=bsb[:],
                     in_=io[BT[L]][m * 128:(m + 1) * 128].unsqueeze(0))
        bT = sp2.tile([1, 128], F32R, tag="bndT", name=f"bnd{L}{m}")
        v.tensor_mul(bT[:], bsb[:], sqvT[:])
        return d_m, bT

    def conv_tile(j, n, wm, bndT):
        """One n-tile of job j: 36 conv matmuls + noise matmul + drain."""
        L, m = jobs[j]
        src_, dst_ = SRC[L], DST[L]
        ps = cps.tile([128, NT, ROUT], F32, tag="cps", name=f"ps{L}{m}{n}")
        first = True
        for k in range(NCH):
            # dy=1 taps first: the start=True matmul must cover the full tile
            for t in (3, 4, 5, 0, 1, 2, 6, 7, 8):
                dy, dx = t // 3, t % 3
                r = n * 8 + dy - 1
                nr, ro = 8, 0
                if r < 0:
                    r, nr, ro = 0, 7, 1
                elif r + 8 > ROUT:
                    nr = ROUT - r
                rhs = src_[k][:, r:r + nr, dx:dx + ROUT].bitcast(F32R)
                te.matmul(ps[:, ro:ro + nr, :], wm[k][:, t, :].bitcast(F32R),
                          rhs, start=first, stop=False)
                first = False
        nz = sp.tile([1, 512], F32R, tag="nz", name=f"nz{L}{m}{n}")
        sy.dma_start(out=nz[:],
                     in_=io[NN[L]][n * 512:(n + 1) * 512].unsqueeze(0))
        te.matmul(ps[:].rearrange("p a b -> p (a b)"),
                  bndT[:].bitcast(F32R), nz[:].bitcast(F32R),
                  start=False, stop=True)
        # drain: dst = max(0.2*P, P)  (two in-place DVE ops, ACT stays free)
        dseg = dst_[m][:, n * 8:n * 8 + 8, 1:1 + ROUT]
        v.tensor_scalar_mul(dseg, ps[:], SLOPE)
        v.tensor_max(dseg, dseg, ps[:])

    prep_dve(0)
    prep_dve(1)

    def zero_borders(t):
        # memset doesn't encode f32r; DMA zeros from DRAM instead
        sy.dma_start(out=t[:, :, 0], in_=io["zz"][:, 0:ROUT])
        sy.dma_start(out=t[:, :, PAD - 1], in_=io["zz"][:, 0:ROUT])

    # ---------------- upsample x into P1 (style scale folded into weights),
    # chunks split across DVE and GpSimd for wall-clock
    for k in range(NCH):
        zero_borders(P1[k])
        xin = wpool.tile([128, RIN, RIN], F32, tag="w", name=f"xin{k}")
        sy.dma_start(out=xin[:], in_=io["x"][k * 128:(k + 1) * 128])
        xh = P2[k][:, 0:ROUT, 0:RIN]
        eng = nc.vector if k % 2 == 0 else nc.gpsimd
        _emit_up2(nc, wpool, ce_sb, co_sb, xin[:], xh,
                  P1[k][:, :, 1:1 + ROUT], 128, eng=eng)

    # rgb upsample: scratch in P2[0], result in P2[3] interior
    rgbin = wpool.tile([RGBC, RIN, RIN], F32, tag="w", name="rgbin")
    sy.dma_start(out=rgbin[:], in_=io["rgb"])
    rgb_xh = P2[0][0:RGBC, 0:ROUT, 0:RIN]
    rgb_up = P2[3][0:RGBC, :, 1:1 + ROUT]
    _emit_up2(nc, wpool, ce_sb, co_sb, rgbin[:], rgb_xh, rgb_up, RGBC,
              eng=nc.gpsimd)
    sy.dma_start(out=io["rgb_up_d"], in_=rgb_up.bitcast(F32))

    # now zero P2 borders (after scratch use)
    for k in range(NCH):
        zero_borders(P2[k])

    # ---------------- the pipelined job loop
    nxt = prep_pe(0)
    for j, (L, m) in enumerate(jobs):
        d_m, bndT = nxt
        wm = wm_j.pop(j)
        conv_tile(j, 0, wm, bndT)
        if j + 2 < len(jobs):
            prep_dve(j + 2)
        for n in range(1, NT):
            conv_tile(j, n, wm, bndT)
        # post-scale chunk m (fold next layer's style scale on layer 1)
        dst_ = DST[L]
        sc.mul(dst_[m][:], dst_[m][:], d_m[:])
        if L == 2:
            sy.dma_start(out=io["ox"][m * 128:(m + 1) * 128, :],
                         in_=dst_[m][:, :, 1:1 + ROUT].bitcast(F32))
        if j + 1 < len(jobs):
            nxt = prep_pe(j + 1)


    # ---------------- layer 3: 1x1 conv to RGB + rgb_up add
    w3 = []
    w3sq = []
    for k in range(NCH):
        t = sp.tile([128, 4], F32R, tag=f"w3_{k}", name=f"w3_{k}")
        sy.dma_start(out=t[:], in_=io["w3t"][k])
        q = sp.tile([128, 4], F32R, tag=f"w3sq_{k}", name=f"w3sq_{k}")
        v.tensor_mul(q[:], t[:], t[:])
        # fold s3 into the conv weights (in place)
        sc.mul(t[:], t[:], s_sb[(3, k)][:])
        w3.append(t)
        w3sq.append(q)
    v3c = sps.tile([4, 2], F32, tag="sps", name="v3c")
    for k in range(NCH):
        te.matmul(v3c[:], w3sq[k][:].bitcast(F32R),
                  ssq_sb[(3, k)][:].bitcast(F32R),
                  start=(k == 0), stop=(k == NCH - 1))
    v3r = sps.tile([1, 4], F32, tag="sps", name="v3r")
    for k in range(NCH):
        te.matmul(v3r[:], ssq_sb[(3, k)][:, 0:1].bitcast(F32R),
                  w3sq[k][:].bitcast(F32R),
                  start=(k == 0), stop=(k == NCH - 1))
    sqv3 = sp.tile([4, 1], F32, tag="sqv3", name="sqv3")
    sc.activation(sqv3[:], v3c[:, 0:1], mybir.ActivationFunctionType.Sqrt,
                  bias=eps_sb[0:4, :], scale=1.0)
    d3 = sp.tile([4, 1], F32, tag="d3", name="d3")
    v.reciprocal(d3[:], sqv3[:])
    sqv3T = sp.tile([1, 4], F32, tag="sqv3T", name="sqv3T")
    sc.activation(sqv3T[:], v3r[:], mybir.ActivationFunctionType.Sqrt,
                  bias=eps_sb[0:1, :], scale=1.0)
    b3sb = sp.tile([1, 4], F32, tag="b3sb", name="b3sb")
    sy.dma_start(out=b3sb[:], in_=io["b3t"].unsqueeze(0))
    bnd3 = sp.tile([1, 4], F32R, tag="bnd3", name="bnd3")
    v.tensor_mul(bnd3[:], b3sb[:], sqv3T[:])

    for n in range(NT):
        ps3 = sps.tile([4, NT, ROUT], F32, tag="sps", name=f"ps3{n}")
        for k in range(NCH):
            rhs = P1[k][:, n * 8:n * 8 + 8, 1:1 + ROUT].bitcast(F32R)
            te.matmul(ps3[:], w3[k][:].bitcast(F32R), rhs,
                      start=(k == 0), stop=False)
        nz3 = wpool.tile([1, 512], F32R, tag="w", name=f"nz3{n}")
        sy.dma_start(out=nz3[:],
                     in_=io["n3"][n * 512:(n + 1) * 512].unsqueeze(0))
        te.matmul(ps3[:].rearrange("p a b -> p (a b)"),
                  bnd3[:].bitcast(F32R), nz3[:].bitcast(F32R),
                  start=False, stop=True)
        ps3f = ps3[0:RGBC].rearrange("p a b -> p (a b)")
        rgbu = wpool.tile([RGBC, 512], F32, tag="w", name=f"rgbu{n}")
        sy.dma_start(out=rgbu[:], in_=io["rgb_up_d"][:, n * 512:(n + 1) * 512])
        tmp = wpool.tile([RGBC, 512], F32, tag="w", name=f"rtmp{n}")
        sc.mul(tmp[:], ps3f, SLOPE)
        qt = wpool.tile([RGBC, 512], F32, tag="w", name=f"rq{n}")
        v.tensor_max(qt[:], tmp[:], ps3f)
        sc.mul(qt[:], qt[:], d3[0:RGBC, :])
        ot = wpool.tile([RGBC, 512], F32, tag="w", name=f"rout{n}")
        v.tensor_add(ot[:], qt[:], rgbu[:])
        sy.dma_start(out=io["orgb"][:, n * 512:(n + 1) * 512], in_=ot[:])


def build():
    nc = bacc.Bacc("TRN2", target_bir_lowering=False, debug=False)
    io = {}

    def inp(name, shape, dt=F32):
        io[name] = nc.dram_tensor(name, list(shape), dt,
                                  kind="ExternalInput").ap()

    inp("x", (C, RIN, RIN))
    inp("rgb", (RGBC, RIN, RIN))
    inp("wv", (C, 2), F32R)
    inp("n1", (NPIX,), F32R)
    inp("n2", (NPIX,), F32R)
    inp("n3", (NPIX,), F32R)
    inp("w1t", (NCH, NCH, 128, 9, 128), F32R)
    inp("w2t", (NCH, NCH, 128, 9, 128), F32R)
    inp("w3t", (NCH, 128, 4), F32R)
    inp("a1t", (2, 128, 8, 128), F32R)
    inp("a2t", (2, 128, 8, 128), F32R)
    inp("a3t", (2, 128, 8, 128), F32R)
    inp("a1b", (C,))
    inp("a2b", (C,))
    inp("a3b", (C,))
    inp("b1t", (C,))
    inp("b2t", (C,))
    inp("b3t", (4,))
    inp("ce", (128, 31))
    inp("co", (128, 31))
    inp("zz", (128, PAD), F32R)
    io["ox"] = nc.dram_tensor("ox", [C, NPIX], F32, kind="ExternalOutput").ap()
    io["orgb"] = nc.dram_tensor("orgb", [RGBC, NPIX], F32,
                                kind="ExternalOutput").ap()
    io["rgb_up_d"] = nc.dram_tensor("rgb_up_d", [RGBC, NPIX], F32).ap()

    with tile.TileContext(nc) as tc:
        with ExitStack() as ctx:
            _emit(ctx, tc, io)
    nc.compile()
    return nc


# ---------------------------------------------------------------- host side
def _pack_at(a_w):
    """A_w [512 out, 512 z] -> [2, 128, 8, 128]: half h, z, block j, i
    where block b = k*4+m holds A_w.T[z-chunk k, i-chunk m]."""
    at = np.ascontiguousarray(a_w.T)                       # [z, i]
    t = at.reshape(4, 128, 4, 128).transpose(0, 2, 1, 3)   # [k, m, z, i]
    t = t.reshape(16, 128, 128)                            # [b, z, i]
    t = t.reshape(2, 8, 128, 128).transpose(0, 2, 1, 3)    # [h, z, j, i]
    return np.ascontiguousarray(t)


def _pack_w(weight):
    """weight [co, ci, 3, 3] -> [m, k, ci(128), t, co(128)] lhsT tiles."""
    t = weight.transpose(1, 2, 3, 0)                       # [ci, ky, kx, co]
    t = t.reshape(C, 9, C)                                 # [ci, t, co]
    t = t.reshape(4, 128, 9, 4, 128).transpose(3, 0, 1, 2, 4)
    return np.ascontiguousarray(t)                         # [m, k, ci, t, co]


def _prep_shared(inputs):
    ce, co = _up_coeffs()
    f = np.float32
    return {
        "w1t": _pack_w(np.asarray(inputs["weight1"], f)),
        "w2t": _pack_w(np.asarray(inputs["weight2"], f)),
        "w3t": np.ascontiguousarray(np.pad(
            np.asarray(inputs["weight3"], f)[:, :, 0, 0].T,
            ((0, 0), (0, 1))).reshape(4, 128, 4)),
        "a1t": _pack_at(np.asarray(inputs["A1_w"], f)),
        "a2t": _pack_at(np.asarray(inputs["A2_w"], f)),
        "a3t": _pack_at(np.asarray(inputs["A3_w"], f)),
        "a1b": np.ascontiguousarray(np.asarray(inputs["A1_b"], f)),
        "a2b": np.ascontiguousarray(np.asarray(inputs["A2_b"], f)),
        "a3b": np.ascontiguousarray(np.asarray(inputs["A3_b"], f)),
        "b1t": np.ascontiguousarray(np.asarray(inputs["B1"], f)),
        "b2t": np.ascontiguousarray(np.asarray(inputs["B2"], f)),
        "b3t": np.ascontiguousarray(np.pad(np.asarray(inputs["B3"], f),
                                           (0, 1))),
        "ce": np.ascontiguousarray(np.tile(ce[None, :], (128, 1))),
        "co": np.ascontiguousarray(np.tile(co[None, :], (128, 1))),
        "zz": np.zeros((128, PAD), np.float32),
    }


def make_in_maps(**inputs):
    shared = _prep_shared(inputs)
    f = np.float32
    x = np.asarray(inputs["x"], f)
    rgb = np.asarray(inputs["rgb"], f)
    w = np.asarray(inputs["w"], f)
    n1 = np.asarray(inputs["noise1"], f).reshape(B_, NPIX)
    n2 = np.asarray(inputs["noise2"], f).reshape(B_, NPIX)
    n3 = np.asarray(inputs["noise3"], f).reshape(B_, NPIX)
    maps = []
    for c in range(B_):
        m = dict(shared)
        m["x"] = np.ascontiguousarray(x[c])
        m["rgb"] = np.ascontiguousarray(rgb[c])
        m["wv"] = np.ascontiguousarray(
            np.stack([w[c], np.zeros(C, np.float32)], axis=1))
        m["n1"] = np.ascontiguousarray(n1[c])
        m["n2"] = np.ascontiguousarray(n2[c])
        m["n3"] = np.ascontiguousarray(n3[c])
        maps.append(m)
    return maps


_NC_CACHE = None


def _get_nc():
    global _NC_CACHE
    if _NC_CACHE is None:
        _NC_CACHE = build()
    return _NC_CACHE


def run(trace=False, **inputs):
    nc = _get_nc()
    in_maps = make_in_maps(**inputs)
    res = run_bass_kernel_spmd(nc, in_maps, list(range(B_)), trace=trace)
    xs = np.stack([res.results[c]["ox"] for c in range(B_)])
    rgbs = np.stack([res.results[c]["orgb"] for c in range(B_)])
    out_x = xs.reshape(B_, C, ROUT, ROUT)
    out_rgb = rgbs.reshape(B_, RGBC, ROUT, ROUT)
    return (out_x, out_rgb), res


def kernel(**inputs):
    (out_x, out_rgb), _ = run(trace=False, **inputs)
    return (out_x, out_rgb)


# revision 25
# speedup vs baseline: 1.0512x; 1.0512x over previous
"""Trainium2 Bass kernel for nn_BlockG (StyleGAN2-ish block G).

Reference math (per sample b):
    x_up  = up2(x)                 # bilinear x2, align_corners
    x1    = leaky(conv_demod(x_up, w, W1, A1, B1, n1))
    x2    = leaky(conv_demod(x1,  w, W2, A2, B2, n2))
    rgb   = up2(rgb) + leaky(conv_demod(x2, w, W3, A3, B3, n3))
    return (x2, rgb)

Key identities used on-device:
  conv(img, W*s[ci]*d[co]) = d[co] * conv(img*s[ci], W)  -> weights shared
  d = rsqrt(Wsq @ s^2 + eps),  Wsq[co,ci] = sum_kk W^2
  leaky(d*P) = d * max(P, 0.2*P)           (d > 0)
  B*noise enters PSUM as a K=1 matmul with lhsT = B/d = B*sqrt(v+eps)

Sharding: pure data-parallel, one sample per NeuronCore (batch 8 = 8 cores).
All matmuls run as float32r (FP22) at full PE rate.
"""

import numpy as np
from contextlib import ExitStack

import concourse.bass as bass
import concourse.bacc as bacc
import concourse.mybir as mybir
import concourse.tile as tile
from concourse.bass_utils import run_bass_kernel_spmd

F32 = mybir.dt.float32
F32R = mybir.dt.float32r

B_ = 8          # batch == n_cores
C = 512         # CIN == COUT == Z
NCH = 4         # channel chunks of 128
RGBC = 3
RIN = 32        # input spatial
ROUT = 64       # output spatial
PAD = 66        # padded spatial
NPIX = ROUT * ROUT   # 4096
NT = 8          # spatial tiles (8 rows x 64 cols = 512 px)
EPS = 1e-8
SLOPE = 0.2


# ---------------------------------------------------------------- upsample
def _up_coeffs():
    """Per-output-position lerp coefficients for 32->64 align_corners x2,
    replicating reference float32 arithmetic.

    even o=2m (m=1..31): out = x[m-1] + (x[m]-x[m-1]) * ce[m-1]
    odd  o=2m+1 (m=0..30): out = x[m]   + (x[m+1]-x[m]) * co[m]
    o=0 -> copy x[0]; o=63 -> copy x[31]
    """
    n, o = RIN, ROUT
    c = np.arange(o, dtype=np.float32) * np.float32((n - 1) / (o - 1))
    i0 = np.floor(c).astype(np.int64)
    t = (c - i0).astype(np.float32)
    ce = np.zeros(31, np.float32)
    co = np.zeros(31, np.float32)
    for m in range(1, 32):
        assert i0[2 * m] == m - 1, (m, i0[2 * m])
        ce[m - 1] = t[2 * m]
    for m in range(0, 31):
        assert i0[2 * m + 1] == m, (m, i0[2 * m + 1])
        co[m] = t[2 * m + 1]
    assert i0[0] == 0 and t[0] == 0.0
    # o=63: c may land exactly on 31 or just below; both reduce to ~x[31].
    assert i0[63] == 31 or (i0[63] == 30 and t[63] > 0.999999)
    return ce, co


def _emit_up2(nc, wpool, ce_sb, co_sb, src_ap, xh_ap, dst_ap, parts, eng=None):
    """Upsample src [parts,32,32] -> dst [parts,64,64] using xh [parts,64,32]
    scratch. dst/xh are arbitrary (possibly strided) SBUF APs."""
    v = eng if eng is not None else nc.vector
    # ---- H axis: src [p,32,32] -> xh [p,64,32]
    dh = wpool.tile([parts, 31, 32], F32, tag="w", name="up_dh")
    v.tensor_sub(dh[:], src_ap[:, 1:32, :], src_ap[:, 0:31, :])
    ceb = ce_sb[0:parts, :].unsqueeze(2).broadcast_to([parts, 31, 32])
    cob = co_sb[0:parts, :].unsqueeze(2).broadcast_to([parts, 31, 32])
    dm = wpool.tile([parts, 31, 32], F32, tag="w", name="up_dm")
    v.tensor_mul(dm[:], dh[:], ceb)
    v.tensor_add(xh_ap[:, 2:64:2, :], dm[:], src_ap[:, 0:31, :])
    dm2 = wpool.tile([parts, 31, 32], F32, tag="w", name="up_dm2")
    v.tensor_mul(dm2[:], dh[:], cob)
    v.tensor_add(xh_ap[:, 1:62:2, :], dm2[:], src_ap[:, 0:31, :])
    v.tensor_copy(xh_ap[:, 0:1, :], src_ap[:, 0:1, :])
    v.tensor_copy(xh_ap[:, 63:64, :], src_ap[:, 31:32, :])
    # ---- W axis: xh [p,64,32] -> dst [p,64,64], two row-halves
    for h2 in range(2):
        xv = xh_ap[:, 32 * h2:32 * h2 + 32, :]
        dv = dst_ap[:, 32 * h2:32 * h2 + 32, :]
        dw = wpool.tile([parts, 32, 31], F32, tag="w", name="up_dw")
        v.tensor_sub(dw[:], xv[:, :, 1:32], xv[:, :, 0:31])
        cwb = ce_sb[0:parts, :].unsqueeze(1).broadcast_to([parts, 32, 31])
        cob2 = co_sb[0:parts, :].unsqueeze(1).broadcast_to([parts, 32, 31])
        wm = wpool.tile([parts, 32, 31], F32, tag="w", name="up_wm")
        v.tensor_mul(wm[:], dw[:], cwb)
        v.tensor_add(dv[:, :, 2:64:2], wm[:], xv[:, :, 0:31])
        wm2 = wpool.tile([parts, 32, 31], F32, tag="w", name="up_wm2")
        v.tensor_mul(wm2[:], dw[:], cob2)
        v.tensor_add(dv[:, :, 1:62:2], wm2[:], xv[:, :, 0:31])
        v.tensor_copy(dv[:, :, 0:1], xv[:, :, 0:1])
        v.tensor_copy(dv[:, :, 63:64], xv[:, :, 31:32])


# ---------------------------------------------------------------- program
def _emit(ctx, tc, io):
    nc = tc.nc
    v, sc, te, sy = nc.vector, nc.scalar, nc.tensor, nc.sync

    # ---------------- pools
    pbuf = ctx.enter_context(tc.tile_pool(name="pbuf", bufs=1))
    P1 = [pbuf.tile([128, ROUT, PAD], F32R, tag=f"P1_{k}", name=f"P1_{k}")
          for k in range(NCH)]
    P2 = [pbuf.tile([128, ROUT, PAD], F32R, tag=f"P2_{k}", name=f"P2_{k}")
          for k in range(NCH)]
    wpool = ctx.enter_context(tc.tile_pool(name="wpool", bufs=14))
    wsqp = ctx.enter_context(tc.tile_pool(name="wsqp", bufs=2))
    sp = ctx.enter_context(tc.tile_pool(name="sp", bufs=1))     # unique tags
    sp2 = ctx.enter_context(tc.tile_pool(name="sp2", bufs=2))   # pipelined tags
    cps = ctx.enter_context(tc.tile_pool(name="cps", bufs=6, space="PSUM"))
    sps = ctx.enter_context(tc.tile_pool(name="sps", bufs=2, space="PSUM"))

    # ---------------- coefficients
    ce_sb = sp.tile([128, 31], F32, tag="ce", name="ce")
    co_sb = sp.tile([128, 31], F32, tag="co", name="co")
    sy.dma_start(out=ce_sb[:], in_=io["ce"])
    sy.dma_start(out=co_sb[:], in_=io["co"])
    eps_sb = sp.tile([128, 1], F32, tag="eps", name="eps")
    v.memset(eps_sb[:], EPS)

    # ---------------- styles for all 3 layers: s = A w + b, ssq = s^2
    wv_sb = []
    for k in range(NCH):
        t = sp.tile([128, 2], F32R, tag=f"wv{k}", name=f"wv{k}")
        sy.dma_start(out=t[:], in_=io["wv"][k * 128:(k + 1) * 128])
        wv_sb.append(t)

    s_sb = {}     # (L, m) -> [128,1]
    ssq_sb = {}   # (L, m) -> [128,1]
    for L, (at_name, ab_name) in enumerate(
            [("a1t", "a1b"), ("a2t", "a2b"), ("a3t", "a3b")], start=1):
        halves = []
        for h in range(2):
            ht = wpool.tile([128, 8, 128], F32R, tag="w", name=f"aT{L}_{h}")
            sy.dma_start(out=ht[:], in_=io[at_name][h])
            halves.append(ht)
        for m in range(NCH):
            ab = sp2.tile([128, 1], F32, tag="ab", name=f"ab{L}{m}")
            sy.dma_start(out=ab[:], in_=io[ab_name][m * 128:(m + 1) * 128])
            s_ps = cps.tile([128, 2], F32, tag="cps", name=f"s_ps{L}{m}")
            for k in range(NCH):
                b = k * 4 + m
                lhsT = halves[b // 8][:, b % 8, :].bitcast(F32R)
                te.matmul(s_ps[:], lhsT, wv_sb[k][:].bitcast(F32R),
                          start=(k == 0), stop=(k == NCH - 1))
            st = sp.tile([128, 1], F32, tag=f"s{L}_{m}", name=f"s{L}_{m}")
            sc.activation(st[:], s_ps[:, 0:1],
                          mybir.ActivationFunctionType.Identity,
                          bias=ab[:], scale=1.0)
            sq = sp.tile([128, 2], F32R, tag=f"ssq{L}_{m}", name=f"ssq{L}_{m}")
            sc.square(sq[:], st[:].broadcast_to([128, 2]))
            s_sb[(L, m)] = st
            ssq_sb[(L, m)] = sq

    # ---------------- conv jobs: layers 1&2 x 4 chunks, software-pipelined.
    # prep_dve(j): weight DMA + ACT squares + DVE reduces -> wsq[j]
    #   (emitted 2 jobs ahead, right after job j-2's first n-tile so the
    #    reduces land ahead of most drains in the DVE queue)
    # prep_pe(j): tiny v/vrow matmuls + sqrt/recip/bnd (emitted just before
    #    job j's conv so the PE hits them with inputs long ready)
    jobs = [(1, m) for m in range(NCH)] + [(2, m) for m in range(NCH)]
    WT = {1: "w1t", 2: "w2t"}
    NN = {1: "n1", 2: "n2"}
    BT = {1: "b1t", 2: "b2t"}
    SRC = {1: P1, 2: P2}
    DST = {1: P2, 2: P1}

    wm_j = {}
    wsq_j = {}

    def prep_dve(j):
        L, m = jobs[j]
        wm = []
        for k in range(NCH):
            t = wpool.tile([128, 9, 128], F32R, tag="w", name=f"w{L}m{m}k{k}")
            sy.dma_start(out=t[:], in_=io[WT[L]][m, k])
            wm.append(t)
        wsq = wsqp.tile([128, NCH, 128], F32R, tag="wsq", name=f"wsq{L}{m}")
        for k in range(NCH):
            sqt = wpool.tile([128, 9, 128], F32, tag="w", name=f"sq{L}{m}{k}")
            sc.square(sqt[:], wm[k][:])
            with nc.allow_low_precision("fp32r matmul operand"):
                v.tensor_reduce(wsq[:, k, :], sqt[:].transpose([0, 2, 1]),
                                axis=mybir.AxisListType.X,
                                op=mybir.AluOpType.add)
            # fold this layer's input-style scale into the conv weights
            # (per-partition ci scale; must happen after the squares)
            sc.mul(wm[k][:], wm[k][:], s_sb[(L, k)][:])
        wm_j[j] = wm
        wsq_j[j] = wsq

    def prep_pe(j):
        L, m = jobs[j]
        wsq = wsq_j[j]
        vcol = sps.tile([128, 2], F32, tag="sps", name=f"vc{L}{m}")
        for k in range(NCH):
            te.matmul(vcol[:], wsq[:, k, :].bitcast(F32R),
                      ssq_sb[(L, k)][:].bitcast(F32R),
                      start=(k == 0), stop=(k == NCH - 1))
        vrow = sps.tile([1, 128], F32, tag="sps", name=f"vr{L}{m}")
        for k in range(NCH):
            te.matmul(vrow[:], ssq_sb[(L, k)][:, 0:1].bitcast(F32R),
                      wsq[:, k, :].bitcast(F32R),
                      start=(k == 0), stop=(k == NCH - 1))
        sqv = sp2.tile([128, 1], F32, tag="sqv", name=f"sqv{L}{m}")
        sc.activation(sqv[:], vcol[:, 0:1], mybir.ActivationFunctionType.Sqrt,
                      bias=eps_sb[:], scale=1.0)
        d_m = sp2.tile([128, 1], F32, tag="d", name=f"d{L}{m}")
        v.reciprocal(d_m[:], sqv[:])
        sqvT = sp2.tile([1, 128], F32, tag="sqvT", name=f"sqvT{L}{m}")
        sc.activation(sqvT[:], vrow[:], mybir.ActivationFunctionType.Sqrt,
                      bias=eps_sb[0:1, :], scale=1.0)
        bsb = sp2.tile([1, 128], F32, tag="bsb", name=f"bsb{L}{m}")
        sy.dma_start(out=bsb[:],
                     in_=io[BT[L]][m * 128:(m + 1) * 128].unsqueeze(0))
        bT = sp2.tile([1, 128], F32R, tag="bndT", name=f"bnd{L}{m}")
        v.tensor_mul(bT[:], bsb[:], sqvT[:])
        return d_m, bT

    def conv_tile(j, n, wm, bndT):
        """One n-tile of job j: 36 conv matmuls + noise matmul + drain."""
        L, m = jobs[j]
        src_, dst_ = SRC[L], DST[L]
        ps = cps.tile([128, NT, ROUT], F32, tag="cps", name=f"ps{L}{m}{n}")
        first = True
        for k in range(NCH):
            # dy=1 taps first: the start=True matmul must cover the full tile
            for t in (3, 4, 5, 0, 1, 2, 6, 7, 8):
                dy, dx = t // 3, t % 3
                r = n * 8 + dy - 1
                nr, ro = 8, 0
                if r < 0:
                    r, nr, ro = 0, 7, 1
                elif r + 8 > ROUT:
                    nr = ROUT - r
                rhs = src_[k][:, r:r + nr, dx:dx + ROUT].bitcast(F32R)
                te.matmul(ps[:, ro:ro + nr, :], wm[k][:, t, :].bitcast(F32R),
                          rhs, start=first, stop=False)
                first = False
        nz = sp.tile([1, 512], F32R, tag="nz", name=f"nz{L}{m}{n}")
        sy.dma_start(out=nz[:],
                     in_=io[NN[L]][n * 512:(n + 1) * 512].unsqueeze(0))
        te.matmul(ps[:].rearrange("p a b -> p (a b)"),
                  bndT[:].bitcast(F32R), nz[:].bitcast(F32R),
                  start=False, stop=True)
        # drain: dst = max(0.2*P, P)  (two in-place DVE ops, ACT stays free)
        dseg = dst_[m][:, n * 8:n * 8 + 8, 1:1 + ROUT]
        v.tensor_scalar_mul(dseg, ps[:], SLOPE)
        v.tensor_max(dseg, dseg, ps[:])

    prep_dve(0)
    prep_dve(1)

    def zero_borders(t):
        # memset doesn't encode f32r; DMA zeros from DRAM instead
        sy.dma_start(out=t[:, :, 0], in_=io["zz"][:, 0:ROUT])
        sy.dma_start(out=t[:, :, PAD - 1], in_=io["zz"][:, 0:ROUT])

    # ---------------- upsample x into P1 (style scale folded into weights),
    # chunks split across DVE and GpSimd for wall-clock
    for k in range(NCH):
        zero_borders(P1[k])
        xin = wpool.tile([128, RIN, RIN], F32, tag="w", name=f"xin{k}")
        sy.dma_start(out=xin[:], in_=io["x"][k * 128:(k + 1) * 128])
        xh = P2[k][:, 0:ROUT, 0:RIN]
        _emit_up2(nc, wpool, ce_sb, co_sb, xin[:], xh,
                  P1[k][:, :, 1:1 + ROUT], 128)

    # rgb upsample: scratch in P2[0], result in P2[3] interior
    rgbin = wpool.tile([RGBC, RIN, RIN], F32, tag="w", name="rgbin")
    sy.dma_start(out=rgbin[:], in_=io["rgb"])
    rgb_xh = P2[0][0:RGBC, 0:ROUT, 0:RIN]
    rgb_up = P2[3][0:RGBC, :, 1:1 + ROUT]
    _emit_up2(nc, wpool, ce_sb, co_sb, rgbin[:], rgb_xh, rgb_up, RGBC)
    sy.dma_start(out=io["rgb_up_d"], in_=rgb_up.bitcast(F32))

    # now zero P2 borders (after scratch use)
    for k in range(NCH):
        zero_borders(P2[k])

    # ---------------- the pipelined job loop
    nxt = prep_pe(0)
    for j, (L, m) in enumerate(jobs):
        d_m, bndT = nxt
        wm = wm_j.pop(j)
        conv_tile(j, 0, wm, bndT)
        if j + 2 < len(jobs):
            prep_dve(j + 2)
        for n in range(1, NT):
            conv_tile(j, n, wm, bndT)
        # post-scale chunk m (fold next layer's style scale on layer 1)
        dst_ = DST[L]
        sc.mul(dst_[m][:], dst_[m][:], d_m[:])
        if L == 2:
            sy.dma_start(out=io["ox"][m * 128:(m + 1) * 128, :],
                         in_=dst_[m][:, :, 1:1 + ROUT].bitcast(F32))
        if j + 1 < len(jobs):
            nxt = prep_pe(j + 1)


    # ---------------- layer 3: 1x1 conv to RGB + rgb_up add
    w3 = []
    w3sq = []
    for k in range(NCH):
        t = sp.tile([128, 4], F32R, tag=f"w3_{k}", name=f"w3_{k}")
        sy.dma_start(out=t[:], in_=io["w3t"][k])
        q = sp.tile([128, 4], F32R, tag=f"w3sq_{k}", name=f"w3sq_{k}")
        v.tensor_mul(q[:], t[:], t[:])
        # fold s3 into the conv weights (in place)
        sc.mul(t[:], t[:], s_sb[(3, k)][:])
        w3.append(t)
        w3sq.append(q)
    v3c = sps.tile([4, 2], F32, tag="sps", name="v3c")
    for k in range(NCH):
        te.matmul(v3c[:], w3sq[k][:].bitcast(F32R),
                  ssq_sb[(3, k)][:].bitcast(F32R),
                  start=(k == 0), stop=(k == NCH - 1))
    v3r = sps.tile([1, 4], F32, tag="sps", name="v3r")
    for k in range(NCH):
        te.matmul(v3r[:], ssq_sb[(3, k)][:, 0:1].bitcast(F32R),
                  w3sq[k][:].bitcast(F32R),
                  start=(k == 0), stop=(k == NCH - 1))
    sqv3 = sp.tile([4, 1], F32, tag="sqv3", name="sqv3")
    sc.activation(sqv3[:], v3c[:, 0:1], mybir.ActivationFunctionType.Sqrt,
                  bias=eps_sb[0:4, :], scale=1.0)
    d3 = sp.tile([4, 1], F32, tag="d3", name="d3")
    v.reciprocal(d3[:], sqv3[:])
    sqv3T = sp.tile([1, 4], F32, tag="sqv3T", name="sqv3T")
    sc.activation(sqv3T[:], v3r[:], mybir.ActivationFunctionType.Sqrt,
                  bias=eps_sb[0:1, :], scale=1.0)
    b3sb = sp.tile([1, 4], F32, tag="b3sb", name="b3sb")
    sy.dma_start(out=b3sb[:], in_=io["b3t"].unsqueeze(0))
    bnd3 = sp.tile([1, 4], F32R, tag="bnd3", name="bnd3")
    v.tensor_mul(bnd3[:], b3sb[:], sqv3T[:])

    for n in range(NT):
        ps3 = sps.tile([4, NT, ROUT], F32, tag="sps", name=f"ps3{n}")
        for k in range(NCH):
            rhs = P1[k][:, n * 8:n * 8 + 8, 1:1 + ROUT].bitcast(F32R)
            te.matmul(ps3[:], w3[k][:].bitcast(F32R), rhs,
                      start=(k == 0), stop=False)
        nz3 = wpool.tile([1, 512], F32R, tag="w", name=f"nz3{n}")
        sy.dma_start(out=nz3[:],
                     in_=io["n3"][n * 512:(n + 1) * 512].unsqueeze(0))
        te.matmul(ps3[:].rearrange("p a b -> p (a b)"),
                  bnd3[:].bitcast(F32R), nz3[:].bitcast(F32R),
                  start=False, stop=True)
        ps3f = ps3[0:RGBC].rearrange("p a b -> p (a b)")
        rgbu = wpool.tile([RGBC, 512], F32, tag="w", name=f"rgbu{n}")
        sy.dma_start(out=rgbu[:], in_=io["rgb_up_d"][:, n * 512:(n + 1) * 512])
        tmp = wpool.tile([RGBC, 512], F32, tag="w", name=f"rtmp{n}")
        sc.mul(tmp[:], ps3f, SLOPE)
        qt = wpool.tile([RGBC, 512], F32, tag="w", name=f"rq{n}")
        v.tensor_max(qt[:], tmp[:], ps3f)
        sc.mul(qt[:], qt[:], d3[0:RGBC, :])
        ot = wpool.tile([RGBC, 512], F32, tag="w", name=f"rout{n}")
        v.tensor_add(ot[:], qt[:], rgbu[:])
        sy.dma_start(out=io["orgb"][:, n * 512:(n + 1) * 512], in_=ot[:])


def build():
    nc = bacc.Bacc("TRN2", target_bir_lowering=False, debug=False)
    io = {}

    def inp(name, shape, dt=F32):
        io[name] = nc.dram_tensor(name, list(shape), dt,
                                  kind="ExternalInput").ap()

    inp("x", (C, RIN, RIN))
    inp("rgb", (RGBC, RIN, RIN))
    inp("wv", (C, 2), F32R)
    inp("n1", (NPIX,), F32R)
    inp("n2", (NPIX,), F32R)
    inp("n3", (NPIX,), F32R)
    inp("w1t", (NCH, NCH, 128, 9, 128), F32R)
    inp("w2t", (NCH, NCH, 128, 9, 128), F32R)
    inp("w3t", (NCH, 128, 4), F32R)
    inp("a1t", (2, 128, 8, 128), F32R)
    inp("a2t", (2, 128, 8, 128), F32R)
    inp("a3t", (2, 128, 8, 128), F32R)
    inp("a1b", (C,))
    inp("a2b", (C,))
    inp("a3b", (C,))
    inp("b1t", (C,))
    inp("b2t", (C,))
    inp("b3t", (4,))
    inp("ce", (128, 31))
    inp("co", (128, 31))
    inp("zz", (128, PAD), F32R)
    io["ox"] = nc.dram_tensor("ox", [C, NPIX], F32, kind="ExternalOutput").ap()
    io["orgb"] = nc.dram_tensor("orgb", [RGBC, NPIX], F32,
                                kind="ExternalOutput").ap()
    io["rgb_up_d"] = nc.dram_tensor("rgb_up_d", [RGBC, NPIX], F32).ap()

    with tile.TileContext(nc) as tc:
        with ExitStack() as ctx:
            _emit(ctx, tc, io)
    nc.compile()
    return nc


# ---------------------------------------------------------------- host side
def _pack_at(a_w):
    """A_w [512 out, 512 z] -> [2, 128, 8, 128]: half h, z, block j, i
    where block b = k*4+m holds A_w.T[z-chunk k, i-chunk m]."""
    at = np.ascontiguousarray(a_w.T)                       # [z, i]
    t = at.reshape(4, 128, 4, 128).transpose(0, 2, 1, 3)   # [k, m, z, i]
    t = t.reshape(16, 128, 128)                            # [b, z, i]
    t = t.reshape(2, 8, 128, 128).transpose(0, 2, 1, 3)    # [h, z, j, i]
    return np.ascontiguousarray(t)


def _pack_w(weight):
    """weight [co, ci, 3, 3] -> [m, k, ci(128), t, co(128)] lhsT tiles."""
    t = weight.transpose(1, 2, 3, 0)                       # [ci, ky, kx, co]
    t = t.reshape(C, 9, C)                                 # [ci, t, co]
    t = t.reshape(4, 128, 9, 4, 128).transpose(3, 0, 1, 2, 4)
    return np.ascontiguousarray(t)                         # [m, k, ci, t, co]


def _prep_shared(inputs):
    ce, co = _up_coeffs()
    f = np.float32
    return {
        "w1t": _pack_w(np.asarray(inputs["weight1"], f)),
        "w2t": _pack_w(np.asarray(inputs["weight2"], f)),
        "w3t": np.ascontiguousarray(np.pad(
            np.asarray(inputs["weight3"], f)[:, :, 0, 0].T,
            ((0, 0), (0, 1))).reshape(4, 128, 4)),
        "a1t": _pack_at(np.asarray(inputs["A1_w"], f)),
        "a2t": _pack_at(np.asarray(inputs["A2_w"], f)),
        "a3t": _pack_at(np.asarray(inputs["A3_w"], f)),
        "a1b": np.ascontiguousarray(np.asarray(inputs["A1_b"], f)),
        "a2b": np.ascontiguousarray(np.asarray(inputs["A2_b"], f)),
        "a3b": np.ascontiguousarray(np.asarray(inputs["A3_b"], f)),
        "b1t": np.ascontiguousarray(np.asarray(inputs["B1"], f)),
        "b2t": np.ascontiguousarray(np.asarray(inputs["B2"], f)),
        "b3t": np.ascontiguousarray(np.pad(np.asarray(inputs["B3"], f),
                                           (0, 1))),
        "ce": np.ascontiguousarray(np.tile(ce[None, :], (128, 1))),
        "co": np.ascontiguousarray(np.tile(co[None, :], (128, 1))),
        "zz": np.zeros((128, PAD), np.float32),
    }


def make_in_maps(**inputs):
    shared = _prep_shared(inputs)
    f = np.float32
    x = np.asarray(inputs["x"], f)
    rgb = np.asarray(inputs["rgb"], f)
    w = np.asarray(inputs["w"], f)
    n1 = np.asarray(inputs["noise1"], f).reshape(B_, NPIX)
    n2 = np.asarray(inputs["noise2"], f).reshape(B_, NPIX)
    n3 = np.asarray(inputs["noise3"], f).reshape(B_, NPIX)
    maps = []
    for c in range(B_):
        m = dict(shared)
        m["x"] = np.ascontiguousarray(x[c])
        m["rgb"] = np.ascontiguousarray(rgb[c])
        m["wv"] = np.ascontiguousarray(
            np.stack([w[c], np.zeros(C, np.float32)], axis=1))
        m["n1"] = np.ascontiguousarray(n1[c])
        m["n2"] = np.ascontiguousarray(n2[c])
        m["n3"] = np.ascontiguousarray(n3[c])
        maps.append(m)
    return maps


_NC_CACHE = None


def _get_nc():
    global _NC_CACHE
    if _NC_CACHE is None:
        _NC_CACHE = build()
    return _NC_CACHE


def run(trace=False, **inputs):
    nc = _get_nc()
    in_maps = make_in_maps(**inputs)
    res = run_bass_kernel_spmd(nc, in_maps, list(range(B_)), trace=trace)
    xs = np.stack([res.results[c]["ox"] for c in range(B_)])
    rgbs = np.stack([res.results[c]["orgb"] for c in range(B_)])
    out_x = xs.reshape(B_, C, ROUT, ROUT)
    out_rgb = rgbs.reshape(B_, RGBC, ROUT, ROUT)
    return (out_x, out_rgb), res


def kernel(**inputs):
    (out_x, out_rgb), _ = run(trace=False, **inputs)
    return (out_x, out_rgb)


# revision 28
# speedup vs baseline: 1.0560x; 1.0046x over previous
"""Trainium2 Bass kernel for nn_BlockG (StyleGAN2-ish block G).

Reference math (per sample b):
    x_up  = up2(x)                 # bilinear x2, align_corners
    x1    = leaky(conv_demod(x_up, w, W1, A1, B1, n1))
    x2    = leaky(conv_demod(x1,  w, W2, A2, B2, n2))
    rgb   = up2(rgb) + leaky(conv_demod(x2, w, W3, A3, B3, n3))
    return (x2, rgb)

Key identities used on-device:
  conv(img, W*s[ci]*d[co]) = d[co] * conv(img*s[ci], W)  -> weights shared
  d = rsqrt(Wsq @ s^2 + eps),  Wsq[co,ci] = sum_kk W^2
  leaky(d*P) = d * max(P, 0.2*P)           (d > 0)
  B*noise enters PSUM as a K=1 matmul with lhsT = B/d = B*sqrt(v+eps)

Sharding: pure data-parallel, one sample per NeuronCore (batch 8 = 8 cores).
All matmuls run as float32r (FP22) at full PE rate.
"""

import numpy as np
from contextlib import ExitStack

import concourse.bass as bass
import concourse.bacc as bacc
import concourse.mybir as mybir
import concourse.tile as tile
from concourse.bass_utils import run_bass_kernel_spmd

F32 = mybir.dt.float32
F32R = mybir.dt.float32r

B_ = 8          # batch == n_cores
C = 512         # CIN == COUT == Z
NCH = 4         # channel chunks of 128
RGBC = 3
RIN = 32        # input spatial
ROUT = 64       # output spatial
PAD = 66        # padded spatial
NPIX = ROUT * ROUT   # 4096
NT = 8          # spatial tiles (8 rows x 64 cols = 512 px)
EPS = 1e-8
SLOPE = 0.2


# ---------------------------------------------------------------- upsample
def _up_coeffs():
    """Per-output-position lerp coefficients for 32->64 align_corners x2,
    replicating reference float32 arithmetic.

    even o=2m (m=1..31): out = x[m-1] + (x[m]-x[m-1]) * ce[m-1]
    odd  o=2m+1 (m=0..30): out = x[m]   + (x[m+1]-x[m]) * co[m]
    o=0 -> copy x[0]; o=63 -> copy x[31]
    """
    n, o = RIN, ROUT
    c = np.arange(o, dtype=np.float32) * np.float32((n - 1) / (o - 1))
    i0 = np.floor(c).astype(np.int64)
    t = (c - i0).astype(np.float32)
    ce = np.zeros(31, np.float32)
    co = np.zeros(31, np.float32)
    for m in range(1, 32):
        assert i0[2 * m] == m - 1, (m, i0[2 * m])
        ce[m - 1] = t[2 * m]
    for m in range(0, 31):
        assert i0[2 * m + 1] == m, (m, i0[2 * m + 1])
        co[m] = t[2 * m + 1]
    assert i0[0] == 0 and t[0] == 0.0
    # o=63: c may land exactly on 31 or just below; both reduce to ~x[31].
    assert i0[63] == 31 or (i0[63] == 30 and t[63] > 0.999999)
    return ce, co


def _emit_up2(nc, wpool, ce_sb, co_sb, src_ap, xh_ap, dst_ap, parts, eng=None):
    """Upsample src [parts,32,32] -> dst [parts,64,64] using xh [parts,64,32]
    scratch. dst/xh are arbitrary (possibly strided) SBUF APs."""
    v = eng if eng is not None else nc.vector
    # ---- H axis: src [p,32,32] -> xh [p,64,32]
    dh = wpool.tile([parts, 31, 32], F32, tag="w", name="up_dh")
    v.tensor_sub(dh[:], src_ap[:, 1:32, :], src_ap[:, 0:31, :])
    ceb = ce_sb[0:parts, :].unsqueeze(2).broadcast_to([parts, 31, 32])
    cob = co_sb[0:parts, :].unsqueeze(2).broadcast_to([parts, 31, 32])
    dm = wpool.tile([parts, 31, 32], F32, tag="w", name="up_dm")
    v.tensor_mul(dm[:], dh[:], ceb)
    v.tensor_add(xh_ap[:, 2:64:2, :], dm[:], src_ap[:, 0:31, :])
    dm2 = wpool.tile([parts, 31, 32], F32, tag="w", name="up_dm2")
    v.tensor_mul(dm2[:], dh[:], cob)
    v.tensor_add(xh_ap[:, 1:62:2, :], dm2[:], src_ap[:, 0:31, :])
    v.tensor_copy(xh_ap[:, 0:1, :], src_ap[:, 0:1, :])
    v.tensor_copy(xh_ap[:, 63:64, :], src_ap[:, 31:32, :])
    # ---- W axis: xh [p,64,32] -> dst [p,64,64], two row-halves
    for h2 in range(2):
        xv = xh_ap[:, 32 * h2:32 * h2 + 32, :]
        dv = dst_ap[:, 32 * h2:32 * h2 + 32, :]
        dw = wpool.tile([parts, 32, 31], F32, tag="w", name="up_dw")
        v.tensor_sub(dw[:], xv[:, :, 1:32], xv[:, :, 0:31])
        cwb = ce_sb[0:parts, :].unsqueeze(1).broadcast_to([parts, 32, 31])
        cob2 = co_sb[0:parts, :].unsqueeze(1).broadcast_to([parts, 32, 31])
        wm = wpool.tile([parts, 32, 31], F32, tag="w", name="up_wm")
        v.tensor_mul(wm[:], dw[:], cwb)
        v.tensor_add(dv[:, :, 2:64:2], wm[:], xv[:, :, 0:31])
        wm2 = wpool.tile([parts, 32, 31], F32, tag="w", name="up_wm2")
        v.tensor_mul(wm2[:], dw[:], cob2)
        v.tensor_add(dv[:, :, 1:62:2], wm2[:], xv[:, :, 0:31])
        v.tensor_copy(dv[:, :, 0:1], xv[:, :, 0:1])
        v.tensor_copy(dv[:, :, 63:64], xv[:, :, 31:32])


# ---------------------------------------------------------------- program
def _emit(ctx, tc, io):
    nc = tc.nc
    v, sc, te, sy = nc.vector, nc.scalar, nc.tensor, nc.sync

    # ---------------- pools
    pbuf = ctx.enter_context(tc.tile_pool(name="pbuf", bufs=1))
    P1 = [pbuf.tile([128, ROUT, PAD], F32R, tag=f"P1_{k}", name=f"P1_{k}")
          for k in range(NCH)]
    P2 = [pbuf.tile([128, ROUT, PAD], F32R, tag=f"P2_{k}", name=f"P2_{k}")
          for k in range(NCH)]
    wpool = ctx.enter_context(tc.tile_pool(name="wpool", bufs=14))
    wsqp = ctx.enter_context(tc.tile_pool(name="wsqp", bufs=2))
    sp = ctx.enter_context(tc.tile_pool(name="sp", bufs=1))     # unique tags
    sp2 = ctx.enter_context(tc.tile_pool(name="sp2", bufs=2))   # pipelined tags
    cps = ctx.enter_context(tc.tile_pool(name="cps", bufs=6, space="PSUM"))
    sps = ctx.enter_context(tc.tile_pool(name="sps", bufs=2, space="PSUM"))

    # ---------------- coefficients
    ce_sb = sp.tile([128, 31], F32, tag="ce", name="ce")
    co_sb = sp.tile([128, 31], F32, tag="co", name="co")
    sy.dma_start(out=ce_sb[:], in_=io["ce"])
    sy.dma_start(out=co_sb[:], in_=io["co"])
    eps_sb = sp.tile([128, 1], F32, tag="eps", name="eps")
    v.memset(eps_sb[:], EPS)

    # ---------------- styles for all 3 layers: s = A w + b, ssq = s^2
    wv_sb = []
    for k in range(NCH):
        t = sp.tile([128, 2], F32R, tag=f"wv{k}", name=f"wv{k}")
        sy.dma_start(out=t[:], in_=io["wv"][k * 128:(k + 1) * 128])
        wv_sb.append(t)

    s_sb = {}     # (L, m) -> [128,1]
    ssq_sb = {}   # (L, m) -> [128,1]
    for L, (at_name, ab_name) in enumerate(
            [("a1t", "a1b"), ("a2t", "a2b"), ("a3t", "a3b")], start=1):
        halves = []
        for h in range(2):
            ht = wpool.tile([128, 8, 128], F32R, tag="w", name=f"aT{L}_{h}")
            sy.dma_start(out=ht[:], in_=io[at_name][h])
            halves.append(ht)
        for m in range(NCH):
            ab = sp2.tile([128, 1], F32, tag="ab", name=f"ab{L}{m}")
            sy.dma_start(out=ab[:], in_=io[ab_name][m * 128:(m + 1) * 128])
            s_ps = cps.tile([128, 2], F32, tag="cps", name=f"s_ps{L}{m}")
            for k in range(NCH):
                b = k * 4 + m
                lhsT = halves[b // 8][:, b % 8, :].bitcast(F32R)
                te.matmul(s_ps[:], lhsT, wv_sb[k][:].bitcast(F32R),
                          start=(k == 0), stop=(k == NCH - 1))
            st = sp.tile([128, 1], F32, tag=f"s{L}_{m}", name=f"s{L}_{m}")
            sc.activation(st[:], s_ps[:, 0:1],
                          mybir.ActivationFunctionType.Identity,
                          bias=ab[:], scale=1.0)
            sq = sp.tile([128, 2], F32R, tag=f"ssq{L}_{m}", name=f"ssq{L}_{m}")
            sc.square(sq[:], st[:].broadcast_to([128, 2]))
            s_sb[(L, m)] = st
            ssq_sb[(L, m)] = sq

    # ---------------- conv jobs: layers 1&2 x 4 chunks, software-pipelined.
    # prep_dve(j): weight DMA + ACT squares + DVE reduces -> wsq[j]
    #   (emitted 2 jobs ahead, right after job j-2's first n-tile so the
    #    reduces land ahead of most drains in the DVE queue)
    # prep_pe(j): tiny v/vrow matmuls + sqrt/recip/bnd (emitted just before
    #    job j's conv so the PE hits them with inputs long ready)
    jobs = [(1, m) for m in range(NCH)] + [(2, m) for m in range(NCH)]
    WT = {1: "w1t", 2: "w2t"}
    NN = {1: "n1", 2: "n2"}
    BT = {1: "b1t", 2: "b2t"}
    SRC = {1: P1, 2: P2}
    DST = {1: P2, 2: P1}

    wm_j = {}
    wsq_j = {}

    def prep_dve(j):
        L, m = jobs[j]
        wm = []
        for k in range(NCH):
            t = wpool.tile([128, 9, 128], F32R, tag="w", name=f"w{L}m{m}k{k}")
            sy.dma_start(out=t[:], in_=io[WT[L]][m, k])
            wm.append(t)
        wsq = wsqp.tile([128, NCH, 128], F32R, tag="wsq", name=f"wsq{L}{m}")
        for k in range(NCH):
            sqt = wpool.tile([128, 9, 128], F32, tag="w", name=f"sq{L}{m}{k}")
            sc.square(sqt[:], wm[k][:])
            with nc.allow_low_precision("fp32r matmul operand"):
                v.tensor_reduce(wsq[:, k, :], sqt[:].transpose([0, 2, 1]),
                                axis=mybir.AxisListType.X,
                                op=mybir.AluOpType.add)
            # fold this layer's input-style scale into the conv weights
            # (per-partition ci scale; must happen after the squares)
            sc.mul(wm[k][:], wm[k][:], s_sb[(L, k)][:])
        wm_j[j] = wm
        wsq_j[j] = wsq

    def prep_pe(j):
        L, m = jobs[j]
        wsq = wsq_j[j]
        vcol = sps.tile([128, 2], F32, tag="sps", name=f"vc{L}{m}")
        for k in range(NCH):
            te.matmul(vcol[:], wsq[:, k, :].bitcast(F32R),
                      ssq_sb[(L, k)][:].bitcast(F32R),
                      start=(k == 0), stop=(k == NCH - 1))
        vrow = sps.tile([1, 128], F32, tag="sps", name=f"vr{L}{m}")
        for k in range(NCH):
            te.matmul(vrow[:], ssq_sb[(L, k)][:, 0:1].bitcast(F32R),
                      wsq[:, k, :].bitcast(F32R),
                      start=(k == 0), stop=(k == NCH - 1))
        sqv = sp2.tile([128, 1], F32, tag="sqv", name=f"sqv{L}{m}")
        sc.activation(sqv[:], vcol[:, 0:1], mybir.ActivationFunctionType.Sqrt,
                      bias=eps_sb[:], scale=1.0)
        d_m = sp2.tile([128, 1], F32, tag="d", name=f"d{L}{m}")
        v.reciprocal(d_m[:], sqv[:])
        sqvT = sp2.tile([1, 128], F32, tag="sqvT", name=f"sqvT{L}{m}")
        sc.activation(sqvT[:], vrow[:], mybir.ActivationFunctionType.Sqrt,
                      bias=eps_sb[0:1, :], scale=1.0)
        bsb = sp2.tile([1, 128], F32, tag="bsb", name=f"bsb{L}{m}")
        sy.dma_start(out=bsb[:],
                     in_=io[BT[L]][m * 128:(m + 1) * 128].unsqueeze(0))
        bT = sp2.tile([1, 128], F32R, tag="bndT", name=f"bnd{L}{m}")
        v.tensor_mul(bT[:], bsb[:], sqvT[:])
        return d_m, bT

    def conv_tile(j, n, wm, bndT):
        """One n-tile of job j: 36 conv matmuls + noise matmul + drain."""
        L, m = jobs[j]
        src_, dst_ = SRC[L], DST[L]
        ps = cps.tile([128, NT, ROUT], F32, tag="cps", name=f"ps{L}{m}{n}")
        first = True
        for k in range(NCH):
            # dy=1 taps first: the start=True matmul must cover the full tile
            for t in (3, 4, 5, 0, 1, 2, 6, 7, 8):
                dy, dx = t // 3, t % 3
                r = n * 8 + dy - 1
                nr, ro = 8, 0
                if r < 0:
                    r, nr, ro = 0, 7, 1
                elif r + 8 > ROUT:
                    nr = ROUT - r
                rhs = src_[k][:, r:r + nr, dx:dx + ROUT].bitcast(F32R)
                te.matmul(ps[:, ro:ro + nr, :], wm[k][:, t, :].bitcast(F32R),
                          rhs, start=first, stop=False)
                first = False
        nz = sp.tile([1, 512], F32R, tag="nz", name=f"nz{L}{m}{n}")
        sy.dma_start(out=nz[:],
                     in_=io[NN[L]][n * 512:(n + 1) * 512].unsqueeze(0))
        te.matmul(ps[:].rearrange("p a b -> p (a b)"),
                  bndT[:].bitcast(F32R), nz[:].bitcast(F32R),
                  start=False, stop=True)
        # drain: dst = max(0.2*P, P)  (two in-place DVE ops, ACT stays free)
        dseg = dst_[m][:, n * 8:n * 8 + 8, 1:1 + ROUT]
        v.tensor_scalar_mul(dseg, ps[:], SLOPE)
        v.tensor_max(dseg, dseg, ps[:])

    def prep_job0():
        # Startup path: DVE is fully busy with the upsample, so compute job
        # 0's demod entirely on the (idle) PE: 9 tap-matmuls per chunk on the
        # squared weights instead of a DVE reduce into wsq.
        L, m = jobs[0]
        wm = []
        for k in range(NCH):
            t = wpool.tile([128, 9, 128], F32R, tag="w", name=f"w{L}m{m}k{k}")
            sy.dma_start(out=t[:], in_=io[WT[L]][m, k])
            wm.append(t)
        vcol = sps.tile([128, 2], F32, tag="sps", name="vc0")
        vrow = sps.tile([1, 128], F32, tag="sps", name="vr0")
        for k in range(NCH):
            sqt = wpool.tile([128, 9, 128], F32R, tag="w", name=f"sq0{k}")
            sc.square(sqt[:], wm[k][:])
            sc.mul(wm[k][:], wm[k][:], s_sb[(L, k)][:])
            for t in range(9):
                te.matmul(vcol[:], sqt[:, t, :],
                          ssq_sb[(L, k)][:].bitcast(F32R),
                          start=(k == 0 and t == 0),
                          stop=(k == NCH - 1 and t == 8))
            for t in range(9):
                te.matmul(vrow[:], ssq_sb[(L, k)][:, 0:1].bitcast(F32R),
                          sqt[:, t, :],
                          start=(k == 0 and t == 0),
                          stop=(k == NCH - 1 and t == 8))
        wm_j[0] = wm
        return vcol, vrow

    def finish_job0(vcol, vrow):
        # DVE pieces of job 0's prep — emitted after the upsample so they
        # don't block it in the DVE queue
        sqv = sp2.tile([128, 1], F32, tag="sqv", name="sqv0")
        sc.activation(sqv[:], vcol[:, 0:1], mybir.ActivationFunctionType.Sqrt,
                      bias=eps_sb[:], scale=1.0)
        d_m = sp2.tile([128, 1], F32, tag="d", name="d0")
        v.reciprocal(d_m[:], sqv[:])
        sqvT = sp2.tile([1, 128], F32, tag="sqvT", name="sqvT0")
        sc.activation(sqvT[:], vrow[:], mybir.ActivationFunctionType.Sqrt,
                      bias=eps_sb[0:1, :], scale=1.0)
        bsb = sp2.tile([1, 128], F32, tag="bsb", name="bsb0")
        sy.dma_start(out=bsb[:], in_=io[BT[1]][0:128].unsqueeze(0))
        bT = sp2.tile([1, 128], F32R, tag="bndT", name="bnd0")
        v.tensor_mul(bT[:], bsb[:], sqvT[:])
        return d_m, bT

    j0_v = prep_job0()

    def zero_borders(t):
        # memset doesn't encode f32r; DMA zeros from DRAM instead
        sy.dma_start(out=t[:, :, 0], in_=io["zz"][:, 0:ROUT])
        sy.dma_start(out=t[:, :, PAD - 1], in_=io["zz"][:, 0:ROUT])

    # ---------------- upsample x into P1 (style scale folded into weights),
    # chunks split across DVE and GpSimd for wall-clock
    for k in range(NCH):
        zero_borders(P1[k])
        xin = wpool.tile([128, RIN, RIN], F32, tag="w", name=f"xin{k}")
        sy.dma_start(out=xin[:], in_=io["x"][k * 128:(k + 1) * 128])
        xh = P2[k][:, 0:ROUT, 0:RIN]
        _emit_up2(nc, wpool, ce_sb, co_sb, xin[:], xh,
                  P1[k][:, :, 1:1 + ROUT], 128)

    # rgb upsample: scratch in P2[0], result in P2[3] interior
    rgbin = wpool.tile([RGBC, RIN, RIN], F32, tag="w", name="rgbin")
    sy.dma_start(out=rgbin[:], in_=io["rgb"])
    rgb_xh = P2[0][0:RGBC, 0:ROUT, 0:RIN]
    rgb_up = P2[3][0:RGBC, :, 1:1 + ROUT]
    _emit_up2(nc, wpool, ce_sb, co_sb, rgbin[:], rgb_xh, rgb_up, RGBC)
    sy.dma_start(out=io["rgb_up_d"], in_=rgb_up.bitcast(F32))

    # now zero P2 borders (after scratch use)
    for k in range(NCH):
        zero_borders(P2[k])

    # ---------------- the pipelined job loop
    nxt = finish_job0(*j0_v)
    prep_dve(1)
    for j, (L, m) in enumerate(jobs):
        d_m, bndT = nxt
        wm = wm_j.pop(j)
        conv_tile(j, 0, wm, bndT)
        if j + 2 < len(jobs):
            prep_dve(j + 2)
        for n in range(1, NT):
            conv_tile(j, n, wm, bndT)
        # post-scale chunk m (fold next layer's style scale on layer 1)
        dst_ = DST[L]
        sc.mul(dst_[m][:], dst_[m][:], d_m[:])
        if L == 2:
            sy.dma_start(out=io["ox"][m * 128:(m + 1) * 128, :],
                         in_=dst_[m][:, :, 1:1 + ROUT].bitcast(F32))
        if j + 1 < len(jobs):
            nxt = prep_pe(j + 1)


    # ---------------- layer 3: 1x1 conv to RGB + rgb_up add
    w3 = []
    w3sq = []
    for k in range(NCH):
        t = sp.tile([128, 4], F32R, tag=f"w3_{k}", name=f"w3_{k}")
        sy.dma_start(out=t[:], in_=io["w3t"][k])
        q = sp.tile([128, 4], F32R, tag=f"w3sq_{k}", name=f"w3sq_{k}")
        v.tensor_mul(q[:], t[:], t[:])
        # fold s3 into the conv weights (in place)
        sc.mul(t[:], t[:], s_sb[(3, k)][:])
        w3.append(t)
        w3sq.append(q)
    v3c = sps.tile([4, 2], F32, tag="sps", name="v3c")
    for k in range(NCH):
        te.matmul(v3c[:], w3sq[k][:].bitcast(F32R),
                  ssq_sb[(3, k)][:].bitcast(F32R),
                  start=(k == 0), stop=(k == NCH - 1))
    v3r = sps.tile([1, 4], F32, tag="sps", name="v3r")
    for k in range(NCH):
        te.matmul(v3r[:], ssq_sb[(3, k)][:, 0:1].bitcast(F32R),
                  w3sq[k][:].bitcast(F32R),
                  start=(k == 0), stop=(k == NCH - 1))
    sqv3 = sp.tile([4, 1], F32, tag="sqv3", name="sqv3")
    sc.activation(sqv3[:], v3c[:, 0:1], mybir.ActivationFunctionType.Sqrt,
                  bias=eps_sb[0:4, :], scale=1.0)
    d3 = sp.tile([4, 1], F32, tag="d3", name="d3")
    v.reciprocal(d3[:], sqv3[:])
    sqv3T = sp.tile([1, 4], F32, tag="sqv3T", name="sqv3T")
    sc.activation(sqv3T[:], v3r[:], mybir.ActivationFunctionType.Sqrt,
                  bias=eps_sb[0:1, :], scale=1.0)
    b3sb = sp.tile([1, 4], F32, tag="b3sb", name="b3sb")
    sy.dma_start(out=b3sb[:], in_=io["b3t"].unsqueeze(0))
    bnd3 = sp.tile([1, 4], F32R, tag="bnd3", name="bnd3")
    v.tensor_mul(bnd3[:], b3sb[:], sqv3T[:])

    for n in range(NT):
        ps3 = sps.tile([4, NT, ROUT], F32, tag="sps", name=f"ps3{n}")
        for k in range(NCH):
            rhs = P1[k][:, n * 8:n * 8 + 8, 1:1 + ROUT].bitcast(F32R)
            te.matmul(ps3[:], w3[k][:].bitcast(F32R), rhs,
                      start=(k == 0), stop=False)
        nz3 = wpool.tile([1, 512], F32R, tag="w", name=f"nz3{n}")
        sy.dma_start(out=nz3[:],
                     in_=io["n3"][n * 512:(n + 1) * 512].unsqueeze(0))
        te.matmul(ps3[:].rearrange("p a b -> p (a b)"),
                  bnd3[:].bitcast(F32R), nz3[:].bitcast(F32R),
                  start=False, stop=True)
        ps3f = ps3[0:RGBC].rearrange("p a b -> p (a b)")
        rgbu = wpool.tile([RGBC, 512], F32, tag="w", name=f"rgbu{n}")
        sy.dma_start(out=rgbu[:], in_=io["rgb_up_d"][:, n * 512:(n + 1) * 512])
        tmp = wpool.tile([RGBC, 512], F32, tag="w", name=f"rtmp{n}")
        sc.mul(tmp[:], ps3f, SLOPE)
        qt = wpool.tile([RGBC, 512], F32, tag="w", name=f"rq{n}")
        v.tensor_max(qt[:], tmp[:], ps3f)
        sc.mul(qt[:], qt[:], d3[0:RGBC, :])
        ot = wpool.tile([RGBC, 512], F32, tag="w", name=f"rout{n}")
        v.tensor_add(ot[:], qt[:], rgbu[:])
        sy.dma_start(out=io["orgb"][:, n * 512:(n + 1) * 512], in_=ot[:])


def build():
    nc = bacc.Bacc("TRN2", target_bir_lowering=False, debug=False)
    io = {}

    def inp(name, shape, dt=F32):
        io[name] = nc.dram_tensor(name, list(shape), dt,
                                  kind="ExternalInput").ap()

    inp("x", (C, RIN, RIN))
    inp("rgb", (RGBC, RIN, RIN))
    inp("wv", (C, 2), F32R)
    inp("n1", (NPIX,), F32R)
    inp("n2", (NPIX,), F32R)
    inp("n3", (NPIX,), F32R)
    inp("w1t", (NCH, NCH, 128, 9, 128), F32R)
    inp("w2t", (NCH, NCH, 128, 9, 128), F32R)
    inp("w3t", (NCH, 128, 4), F32R)
    inp("a1t", (2, 128, 8, 128), F32R)
    inp("a2t", (2, 128, 8, 128), F32R)
    inp("a3t", (2, 128, 8, 128), F32R)
    inp("a1b", (C,))
    inp("a2b", (C,))
    inp("a3b", (C,))
    inp("b1t", (C,))
    inp("b2t", (C,))
    inp("b3t", (4,))
    inp("ce", (128, 31))
    inp("co", (128, 31))
    inp("zz", (128, PAD), F32R)
    io["ox"] = nc.dram_tensor("ox", [C, NPIX], F32, kind="ExternalOutput").ap()
    io["orgb"] = nc.dram_tensor("orgb", [RGBC, NPIX], F32,
                                kind="ExternalOutput").ap()
    io["rgb_up_d"] = nc.dram_tensor("rgb_up_d", [RGBC, NPIX], F32).ap()

    with tile.TileContext(nc) as tc:
        with ExitStack() as ctx:
            _emit(ctx, tc, io)
    nc.compile()
    return nc


# ---------------------------------------------------------------- host side
def _pack_at(a_w):
    """A_w [512 out, 512 z] -> [2, 128, 8, 128]: half h, z, block j, i
    where block b = k*4+m holds A_w.T[z-chunk k, i-chunk m]."""
    at = np.ascontiguousarray(a_w.T)                       # [z, i]
    t = at.reshape(4, 128, 4, 128).transpose(0, 2, 1, 3)   # [k, m, z, i]
    t = t.reshape(16, 128, 128)                            # [b, z, i]
    t = t.reshape(2, 8, 128, 128).transpose(0, 2, 1, 3)    # [h, z, j, i]
    return np.ascontiguousarray(t)


def _pack_w(weight):
    """weight [co, ci, 3, 3] -> [m, k, ci(128), t, co(128)] lhsT tiles."""
    t = weight.transpose(1, 2, 3, 0)                       # [ci, ky, kx, co]
    t = t.reshape(C, 9, C)                                 # [ci, t, co]
    t = t.reshape(4, 128, 9, 4, 128).transpose(3, 0, 1, 2, 4)
    return np.ascontiguousarray(t)                         # [m, k, ci, t, co]


def _prep_shared(inputs):
    ce, co = _up_coeffs()
    f = np.float32
    return {
        "w1t": _pack_w(np.asarray(inputs["weight1"], f)),
        "w2t": _pack_w(np.asarray(inputs["weight2"], f)),
        "w3t": np.ascontiguousarray(np.pad(
            np.asarray(inputs["weight3"], f)[:, :, 0, 0].T,
            ((0, 0), (0, 1))).reshape(4, 128, 4)),
        "a1t": _pack_at(np.asarray(inputs["A1_w"], f)),
        "a2t": _pack_at(np.asarray(inputs["A2_w"], f)),
        "a3t": _pack_at(np.asarray(inputs["A3_w"], f)),
        "a1b": np.ascontiguousarray(np.asarray(inputs["A1_b"], f)),
        "a2b": np.ascontiguousarray(np.asarray(inputs["A2_b"], f)),
        "a3b": np.ascontiguousarray(np.asarray(inputs["A3_b"], f)),
        "b1t": np.ascontiguousarray(np.asarray(inputs["B1"], f)),
        "b2t": np.ascontiguousarray(np.asarray(inputs["B2"], f)),
        "b3t": np.ascontiguousarray(np.pad(np.asarray(inputs["B3"], f),
                                           (0, 1))),
        "ce": np.ascontiguousarray(np.tile(ce[None, :], (128, 1))),
        "co": np.ascontiguousarray(np.tile(co[None, :], (128, 1))),
        "zz": np.zeros((128, PAD), np.float32),
    }


def make_in_maps(**inputs):
    shared = _prep_shared(inputs)
    f = np.float32
    x = np.asarray(inputs["x"], f)
    rgb = np.asarray(inputs["rgb"], f)
    w = np.asarray(inputs["w"], f)
    n1 = np.asarray(inputs["noise1"], f).reshape(B_, NPIX)
    n2 = np.asarray(inputs["noise2"], f).reshape(B_, NPIX)
    n3 = np.asarray(inputs["noise3"], f).reshape(B_, NPIX)
    maps = []
    for c in range(B_):
        m = dict(shared)
        m["x"] = np.ascontiguousarray(x[c])
        m["rgb"] = np.ascontiguousarray(rgb[c])
        m["wv"] = np.ascontiguousarray(
            np.stack([w[c], np.zeros(C, np.float32)], axis=1))
        m["n1"] = np.ascontiguousarray(n1[c])
        m["n2"] = np.ascontiguousarray(n2[c])
        m["n3"] = np.ascontiguousarray(n3[c])
        maps.append(m)
    return maps


_NC_CACHE = None


def _get_nc():
    global _NC_CACHE
    if _NC_CACHE is None:
        _NC_CACHE = build()
    return _NC_CACHE


def run(trace=False, **inputs):
    nc = _get_nc()
    in_maps = make_in_maps(**inputs)
    res = run_bass_kernel_spmd(nc, in_maps, list(range(B_)), trace=trace)
    xs = np.stack([res.results[c]["ox"] for c in range(B_)])
    rgbs = np.stack([res.results[c]["orgb"] for c in range(B_)])
    out_x = xs.reshape(B_, C, ROUT, ROUT)
    out_rgb = rgbs.reshape(B_, RGBC, ROUT, ROUT)
    return (out_x, out_rgb), res


def kernel(**inputs):
    (out_x, out_rgb), _ = run(trace=False, **inputs)
    return (out_x, out_rgb)
